# revision 19
# baseline (speedup 1.0000x reference)
"""Multi-head self-attention (B=8, N=1024, C=768, H=12, D=64) on 8 Trainium2
NeuronCores, batch-parallel (one batch element per core).

Per-core dataflow (activations kept feature-major, "T" = [feature, token]):
  xT [768,1024] --(PE)--> QT,KT [768,1024] (d-major) and V [1024,768+ones]
  S^T[k,q] = KT_h-slice^T x QT_h      (K=d=64; two heads of a pair via
                                       PE row-tiling at partitions 0/64)
  E = exp(S^T * scale) -> fp16        (ACT; no max-subtract: |S*scale| < 9,
                                       so exp < 6e3 fits fp16 with margin)
  ctxU^T[d,q] (+denominator row) = V_ext_h^T x E   (ones column in V gives
                                                    the softmax denominator)
  evac ctxU fast (frees PSUM); one pair behind: reciprocal + masked K=1
  ones-matmul broadcast + one in-place multiply normalizes the pair.
  out[q,o] = ctxN^T-slices^T x wpT + bias(bcast, DVE add)

The qkv projections run float32r (fp22 multiply, fp32 accumulate, 2 PE
cycles/col). Q/K/V/E/ctx/proj-weight storage is fp16 (e5m10): it streams at
1 PE cycle/col like bf16 but with 8x finer mantissa, halving the S and PV
matmul time at ~5e-4 relative error.
"""
import numpy as np

import concourse.bass as bass
import concourse.tile as tile
from concourse import bacc, mybir
from concourse.bass_utils import run_bass_kernel_spmd

N_CORES = 8
N = 1024          # tokens per core (batch element)
C = 768           # model dim
H = 12            # heads
D = 64            # head dim
SCALE = D ** -0.5
NT = N // 128     # 8 token tiles
CT = C // 128     # 6 feature tiles
F32 = mybir.dt.float32
F32R = mybir.dt.float32r
BF16 = mybir.dt.bfloat16
FP16 = mybir.dt.float16
EXP = mybir.ActivationFunctionType.Exp

QK_BF16 = False   # False: keep the Q/K path (C-phase + S matmuls) in fp32r


def _r(ap):
    return ap.bitcast(F32R)


def build():
    nc = bacc.Bacc(
        "TRN2", target_bir_lowering=False, debug=False, num_devices=N_CORES
    )
    xT_d = nc.dram_tensor("xT", [C, N], FP16, kind="ExternalInput").ap()
    wqT_d = nc.dram_tensor("wqT", [C, 3 * C], FP16, kind="ExternalInput").ap()
    wpT_d = nc.dram_tensor("wpT", [C, C], FP16, kind="ExternalInput").ap()
    bias_d = nc.dram_tensor("bias_bc", [128, C], F32, kind="ExternalInput").ap()
    ones_d = nc.dram_tensor("ones_v", [128, NT * H], FP16, kind="ExternalInput").ap()
    onesr_d = nc.dram_tensor("ones_mask", [2, 128], FP16, kind="ExternalInput").ap()
    out_d = nc.dram_tensor("out", [N, C], F32, kind="ExternalOutput").ap()

    qk_dt = FP16

    with tile.TileContext(nc) as tc:
        with (
            tc.tile_pool(name="big", bufs=1) as big,
            tc.tile_pool(name="wqk", bufs=3) as wqkp,
            tc.tile_pool(name="e", bufs=4) as ep,
            tc.tile_pool(name="outb", bufs=2) as outp,
            tc.tile_pool(name="norm", bufs=2) as normp,
            tc.tile_pool(name="psA", bufs=2, space="PSUM") as psA,
            tc.tile_pool(name="psC", bufs=2, space="PSUM") as psC,
        ):
            # ---- persistent SBUF tensors -------------------------------
            xqk = big.tile([128, CT, N], FP16, name="xqk", tag="xqk")
            wvs = big.tile([128, CT, C], FP16, name="wvs", tag="wvs")
            wps = big.tile([128, CT, C], FP16, name="wps", tag="wps")
            QT = big.tile([128, CT, N], qk_dt, name="QT", tag="QT")
            KT = big.tile([128, CT, N], qk_dt, name="KT", tag="KT")
            V = big.tile([128, NT, H * (D + 1)], FP16, name="V", tag="V")
            ctxN = big.tile([128, CT, N], FP16, name="ctxN", tag="ctxN")
            bias_sb = big.tile([128, C], F32, name="bias_sb", tag="bias")
            ones_mask = [
                big.tile([1, 128], FP16, name=f"ones_mask{i}", tag=f"onesr{i}")
                for i in range(2)
            ]

            # Batched input DMAs: each dma_start costs ~600ns of issue time on
            # its queue engine, so coarse 3D strided transfers beat per-tile
            # loads.  x/wv split in ct halves so phase B can start early.
            for i in range(2):
                nc.sync.dma_start(ones_mask[i][:], onesr_d[i:i + 1, :])
            xT_r = xT_d.rearrange("(ct p) n -> p ct n", p=128)
            wv_r = wqT_d[:, 2 * C:3 * C].rearrange("(ct p) n -> p ct n", p=128)
            hh = CT // 2
            nc.sync.dma_start(xqk[:, 0:hh, :], xT_r[:, 0:hh, :])
            nc.sync.dma_start(wvs[:, 0:hh, :], wv_r[:, 0:hh, :])
            nc.sync.dma_start(xqk[:, hh:CT, :], xT_r[:, hh:CT, :])
            nc.sync.dma_start(wvs[:, hh:CT, :], wv_r[:, hh:CT, :])
            nc.sync.dma_start(bias_sb[:], bias_d[:])
            v_ones = V[:].rearrange("p nt (h e) -> p (nt h) e", e=D + 1)
            nc.gpsimd.dma_start(
                v_ones[:, :, D:D + 1],
                ones_d.rearrange("p (m o) -> p m o", o=1),
            )

            # ---- phase B: V (token-major, bf16) ------------------------
            for nt in range(NT):
                pv = psA.tile([128, N], F32, tag="ps", name=f"pv{nt}")
                for ct in range(CT):
                    lhsT = xqk[:, ct, nt * 128:(nt + 1) * 128]
                    for lo, w in ((0, 512), (512, 256)):
                        nc.tensor.matmul(
                            pv[:, lo:lo + w],
                            lhsT,
                            wvs[:, ct, lo:lo + w],
                            start=(ct == 0),
                            stop=(ct == CT - 1),
                        )
                vt = V[:, nt, :].rearrange("p (h e) -> p h e", e=D + 1)
                nc.scalar.copy(
                    vt[:, :, 0:D], pv[:, 0:C].rearrange("p (h d) -> p h d", d=D)
                )

            # ---- phase C: QT / KT (feature-major) ----------------------
            # one batched [128, CT, 128] weight DMA per (jt, base) group,
            # issued from the gpsimd queue to keep the sync queue free
            for jt in range(CT):
                for base, dst in ((0, QT), (C, KT)):
                    wg = wqkp.tile(
                        [128, CT, 128], FP16, tag="wqk", name=f"w{base}_{jt}"
                    )
                    src = wqT_d[:, base + jt * 128:base + (jt + 1) * 128]
                    nc.gpsimd.dma_start(
                        wg[:], src.rearrange("(ct p) m -> p ct m", p=128)
                    )
                    ps = psA.tile([128, N], F32, tag="ps", name=f"q{base}_{jt}")
                    for ct in range(CT):
                        for qc in range(2):
                            nc.tensor.matmul(
                                ps[:, qc * 512:(qc + 1) * 512],
                                wg[:, ct, :],
                                xqk[:, ct, qc * 512:(qc + 1) * 512],
                                start=(ct == 0),
                                stop=(ct == CT - 1),
                            )
                    nc.vector.tensor_copy(dst[:, jt, :], ps[:])

            # proj weights are first needed far later; load them now so the
            # casting DMAs do not delay the startup x/w loads
            nc.sync.dma_start(
                wps[:], wpT_d.rearrange("(ct p) n -> p ct n", p=128)
            )

            # ---- phase D: attention, head pairs, row-packed S ----------
            deferred_norm = []

            def emit_norm(jobs):
                # jobs = per-pair (rc_h0, rc_h1, p): convert each [1, N]
                # reciprocal row to fp16, broadcast to [128, N] with two K=1
                # masked fp16 ones-matmuls per 512-col chunk, then normalize
                # the pair with one fp16 multiply.
                for rcs_, p_ in jobs:
                    rcrs = []
                    for half in range(2):
                        rcr = normp.tile(
                            [1, N], FP16, tag="rcr", name=f"rcr{2 * p_ + half}", bufs=4
                        )
                        nc.vector.tensor_copy(rcr[:], rcs_[half][:])
                        rcrs.append(rcr)
                    bc_ps = psA.tile([128, N], F32, tag="ps", name=f"bcp{p_}")
                    for qc in range(2):
                        for half in range(2):
                            nc.tensor.matmul(
                                bc_ps[:, qc * 512:(qc + 1) * 512],
                                ones_mask[half][:],
                                rcrs[half][:, qc * 512:(qc + 1) * 512],
                                start=(half == 0),
                                stop=(half == 1),
                            )
                    bc = normp.tile([128, N], FP16, tag="bc", name=f"bc{p_}", bufs=1)
                    nc.vector.tensor_copy(bc[:], bc_ps[:])
                    nc.vector.tensor_mul(ctxN[:, p_, :], ctxN[:, p_, :], bc[:])

            # Software pipeline across head pairs: during pair p's S/exp
            # stream (ACT-paced), the PE executes pair p-1's PV matmuls,
            # whose E tiles are already complete. PV then never waits on the
            # in-flight exp, and attention runs at the ACT exp rate.
            def emit_pv(pcps, pes, pp, kt):
                for half in range(2):
                    h = 2 * pp + half
                    for qc in range(2):
                        nc.tensor.matmul(
                            pcps[half][:, qc * 512:(qc + 1) * 512],
                            V[:, kt, h * (D + 1):(h + 1) * (D + 1)],
                            pes[kt][half][:, qc * 512:(qc + 1) * 512],
                            start=(kt == 0),
                            stop=(kt == NT - 1),
                        )

            def emit_evac(pcps, pp, on_act=False):
                # both PSUM-freeing evacs first; the reciprocal of the
                # denominator row is taken straight from PSUM behind them
                # (off ACT so the exp pacer stays clean, and after the evacs
                # so the ctx-bank handover is not delayed). In the drain the
                # exp stream is over, so the evacs go to ACT to run in
                # parallel with the DVE reciprocals.
                for half in range(2):
                    po = half * 64
                    if on_act:
                        nc.scalar.copy(ctxN[po:po + 64, pp, :], pcps[half][0:D, :])
                    else:
                        nc.vector.tensor_copy(
                            ctxN[po:po + 64, pp, :], pcps[half][0:D, :]
                        )
                rcs = []
                for half in range(2):
                    rc = normp.tile(
                        [1, N], F32, tag="den", name=f"rc{2 * pp + half}", bufs=4
                    )
                    nc.vector.reciprocal_approx_fast(rc[:], pcps[half][D:D + 1, :])
                    rcs.append(rc)
                deferred_norm.append((rcs, pp))

            prev = None
            for p in range(CT):  # 6 head pairs; pair p = heads (2p, 2p+1)
                cps = [
                    psC.tile([D + 1, N], F32, tag="ctx", name=f"ctx{2 * p + i}")
                    for i in range(2)
                ]
                es = []
                for kt in range(NT):
                    sps = [
                        psA.tile([128, N], F32, tag="ps", name=f"s{2 * p + i}_{kt}")
                        for i in range(2)
                    ]
                    # interleave the two 64-row tile_position halves so their
                    # matmuls run concurrently on the row-tiled PE array
                    for qc in range(2):
                        for half in range(2):
                            po = half * 64
                            nc.tensor.matmul(
                                sps[half][:, qc * 512:(qc + 1) * 512],
                                KT[po:po + 64, p, kt * 128:(kt + 1) * 128],
                                QT[po:po + 64, p, qc * 512:(qc + 1) * 512],
                                start=True,
                                stop=True,
                                tile_position=(po, 0),
                            )
                    row = []
                    for half in range(2):
                        h = 2 * p + half
                        e = ep.tile(
                            [128, N], FP16, tag="e", name=f"e{h}_{kt}", bufs=12
                        )
                        nc.scalar.activation(e[:], sps[half][:], EXP, scale=SCALE)
                        row.append(e)
                    es.append(row)
                    if prev is not None:
                        emit_pv(prev[0], prev[1], prev[2], kt)
                    if kt == 1 and deferred_norm:
                        # normalize the pair before last while streams run
                        emit_norm(deferred_norm)
                        deferred_norm = []
                if prev is not None:
                    emit_evac(prev[0], prev[2])
                prev = (cps, es, p)
            # drain: norm the second-to-last pair first (its reciprocals are
            # ready), then PV + evac for the final pair
            emit_norm(deferred_norm)
            deferred_norm = []
            for kt in range(NT):
                emit_pv(prev[0], prev[1], prev[2], kt)
            emit_evac(prev[0], prev[2], on_act=True)
            last = prev[2]

            # ---- phase E: output projection + bias ---------------------
            # ct outer, lo inner: each ctxN stationary chunk is loaded once.
            # The last pair's ct-chunk is accumulated LAST, and its norm
            # (broadcast matmul + multiply) is emitted after nt=0's first
            # chunks, so the PE projects the already-normalized pairs while
            # the last pair's normalization chain completes.
            cts = [ct for ct in range(CT) if ct != last] + [last]
            for nt in range(NT):
                ps = psA.tile([128, N], F32, tag="ps", name=f"po{nt}")
                for i, ct in enumerate(cts[:-1]):
                    for lo, w in ((0, 512), (512, 256)):
                        nc.tensor.matmul(
                            ps[:, lo:lo + w],
                            ctxN[:, ct, nt * 128:(nt + 1) * 128],
                            wps[:, ct, lo:lo + w],
                            start=(i == 0),
                            stop=False,
                        )
                if nt == 0 and deferred_norm:
                    emit_norm(deferred_norm)
                    deferred_norm = []
                for lo, w in ((0, 512), (512, 256)):
                    nc.tensor.matmul(
                        ps[:, lo:lo + w],
                        ctxN[:, last, nt * 128:(nt + 1) * 128],
                        wps[:, last, lo:lo + w],
                        start=False,
                        stop=True,
                    )
                ob = outp.tile([128, C], F32, tag="ob", name=f"ob{nt}")
                nc.vector.tensor_add(ob[:], ps[:, 0:C], bias_sb[:])
                nc.sync.dma_start(out_d[nt * 128:(nt + 1) * 128, :], ob[:])

    nc.compile()
    return nc


_CACHE = {}


def _get_nc():
    if "nc" not in _CACHE:
        _CACHE["nc"] = build()
    return _CACHE["nc"]


def run(inputs, trace=False):
    """Run on hardware; returns (full_output [8,1024,768] f32, BassKernelResults)."""
    nc = _get_nc()
    x = np.asarray(inputs["x"], dtype=np.float32)
    w_qkv = np.asarray(inputs["w_qkv"], dtype=np.float32)
    w_proj = np.asarray(inputs["w_proj"], dtype=np.float32)
    b_proj = np.asarray(inputs["b_proj"], dtype=np.float32)

    xT = np.ascontiguousarray(x.transpose(0, 2, 1)).astype(np.float16)
    wqT = np.ascontiguousarray(w_qkv.T).astype(np.float16)
    wpT = np.ascontiguousarray(w_proj.T).astype(np.float16)
    bias_bc = np.ascontiguousarray(np.broadcast_to(b_proj.reshape(1, C), (128, C)))
    ones_v = np.ones((128, NT * H), dtype=np.float16)

    in_maps = [
        {
            "xT": xT[b],
            "wqT": wqT,
            "wpT": wpT,
            "bias_bc": bias_bc,
            "ones_v": ones_v,
            "ones_mask": np.kron(np.eye(2), np.ones((1, 64))).astype(np.float16),
        }
        for b in range(N_CORES)
    ]
    res = run_bass_kernel_spmd(nc, in_maps, list(range(N_CORES)), trace=trace)
    out = np.stack([res.results[b]["out"] for b in range(N_CORES)])
    return out, res


def kernel(x, w_qkv, w_proj, b_proj):
    out, _ = run(
        {"x": x, "w_qkv": w_qkv, "w_proj": w_proj, "b_proj": b_proj}, trace=False
    )
    return out



# revision 23
# speedup vs baseline: 1.0163x; 1.0163x over previous
"""Multi-head self-attention (B=8, N=1024, C=768, H=12, D=64) on 8 Trainium2
NeuronCores, batch-parallel (one batch element per core).

Per-core dataflow (activations kept feature-major, "T" = [feature, token]):
  xT [768,1024] --(PE)--> QT,KT [768,1024] (d-major) and V [1024,768+ones]
  S^T[k,q] = KT_h-slice^T x QT_h      (K=d=64; two heads of a pair via
                                       PE row-tiling at partitions 0/64)
  E = exp(S^T * scale) -> fp16        (ACT; no max-subtract: |S*scale| < 9,
                                       so exp < 6e3 fits fp16 with margin)
  ctxU^T[d,q] (+denominator row) = V_ext_h^T x E   (ones column in V gives
                                                    the softmax denominator)
  evac ctxU fast (frees PSUM); one pair behind: reciprocal + masked K=1
  ones-matmul broadcast + one in-place multiply normalizes the pair.
  out[q,o] = ctxN^T-slices^T x wpT + bias(bcast, DVE add)

The qkv projections run float32r (fp22 multiply, fp32 accumulate, 2 PE
cycles/col). Q/K/V/E/ctx/proj-weight storage is fp16 (e5m10): it streams at
1 PE cycle/col like bf16 but with 8x finer mantissa, halving the S and PV
matmul time at ~5e-4 relative error.
"""
import numpy as np

import concourse.bass as bass
import concourse.tile as tile
from concourse import bacc, mybir
from concourse.bass_utils import run_bass_kernel_spmd

N_CORES = 8
N = 1024          # tokens per core (batch element)
C = 768           # model dim
H = 12            # heads
D = 64            # head dim
SCALE = D ** -0.5
NT = N // 128     # 8 token tiles
CT = C // 128     # 6 feature tiles
F32 = mybir.dt.float32
F32R = mybir.dt.float32r
BF16 = mybir.dt.bfloat16
FP16 = mybir.dt.float16
EXP = mybir.ActivationFunctionType.Exp

QK_BF16 = False   # False: keep the Q/K path (C-phase + S matmuls) in fp32r


def _r(ap):
    return ap.bitcast(F32R)


def build():
    nc = bacc.Bacc(
        "TRN2", target_bir_lowering=False, debug=False, num_devices=N_CORES
    )
    xT_d = nc.dram_tensor("xT", [C, N], FP16, kind="ExternalInput").ap()
    wqT_d = nc.dram_tensor("wqT", [C, 3 * C], FP16, kind="ExternalInput").ap()
    wpT_d = nc.dram_tensor("wpT", [C, C], FP16, kind="ExternalInput").ap()
    bias_d = nc.dram_tensor("bias_bc", [128, C], F32, kind="ExternalInput").ap()
    onesr_d = nc.dram_tensor("ones_mask", [2, 128], FP16, kind="ExternalInput").ap()
    out_d = nc.dram_tensor("out", [N, C], F32, kind="ExternalOutput").ap()

    qk_dt = FP16

    with tile.TileContext(nc) as tc:
        with (
            tc.tile_pool(name="big", bufs=1) as big,
            tc.tile_pool(name="wqk", bufs=3) as wqkp,
            tc.tile_pool(name="e", bufs=4) as ep,
            tc.tile_pool(name="outb", bufs=2) as outp,
            tc.tile_pool(name="norm", bufs=2) as normp,
            tc.tile_pool(name="psA", bufs=2, space="PSUM") as psA,
            tc.tile_pool(name="psC", bufs=2, space="PSUM") as psC,
        ):
            # ---- persistent SBUF tensors -------------------------------
            xqk = big.tile([128, CT, N], FP16, name="xqk", tag="xqk")
            wvs = big.tile([128, CT, C], FP16, name="wvs", tag="wvs")
            wps = big.tile([128, CT, C], FP16, name="wps", tag="wps")
            QT = big.tile([128, CT, N], qk_dt, name="QT", tag="QT")
            KT = big.tile([128, CT, N], qk_dt, name="KT", tag="KT")
            V = big.tile([128, NT, H * (D + 1)], FP16, name="V", tag="V")
            ctxN = big.tile([128, CT, N], FP16, name="ctxN", tag="ctxN")
            bias_sb = big.tile([128, C], F32, name="bias_sb", tag="bias")
            ones_mask = [
                big.tile([1, 128], FP16, name=f"ones_mask{i}", tag=f"onesr{i}")
                for i in range(2)
            ]

            # Batched input DMAs: each dma_start costs ~600ns of issue time on
            # its queue engine, so coarse 3D strided transfers beat per-tile
            # loads.  x/wv split in ct halves so phase B can start early.
            for i in range(2):
                nc.sync.dma_start(ones_mask[i][:], onesr_d[i:i + 1, :])
            xT_r = xT_d.rearrange("(ct p) n -> p ct n", p=128)
            wv_r = wqT_d[:, 2 * C:3 * C].rearrange("(ct p) n -> p ct n", p=128)
            hh = CT // 2
            nc.sync.dma_start(xqk[:, 0:hh, :], xT_r[:, 0:hh, :])
            nc.sync.dma_start(wvs[:, 0:hh, :], wv_r[:, 0:hh, :])
            nc.sync.dma_start(xqk[:, hh:CT, :], xT_r[:, hh:CT, :])
            nc.sync.dma_start(wvs[:, hh:CT, :], wv_r[:, hh:CT, :])
            nc.sync.dma_start(bias_sb[:], bias_d[:])
            v_ones = V[:].rearrange("p nt (h e) -> p (nt h) e", e=D + 1)
            nc.gpsimd.memset(v_ones[:, :, D:D + 1], 1.0)

            # ---- phase B: V (token-major, bf16) ------------------------
            for nt in range(NT):
                pv = psA.tile([128, N], F32, tag="ps", name=f"pv{nt}")
                for ct in range(CT):
                    lhsT = xqk[:, ct, nt * 128:(nt + 1) * 128]
                    for lo, w in ((0, 512), (512, 256)):
                        nc.tensor.matmul(
                            pv[:, lo:lo + w],
                            lhsT,
                            wvs[:, ct, lo:lo + w],
                            start=(ct == 0),
                            stop=(ct == CT - 1),
                        )
                vt = V[:, nt, :].rearrange("p (h e) -> p h e", e=D + 1)
                nc.scalar.copy(
                    vt[:, :, 0:D], pv[:, 0:C].rearrange("p (h d) -> p h d", d=D)
                )

            # ---- phase C: QT / KT (feature-major) ----------------------
            # one batched [128, CT, 128] weight DMA per (jt, base) group,
            # issued from the gpsimd queue to keep the sync queue free
            for jt in range(CT):
                for base, dst in ((0, QT), (C, KT)):
                    wg = wqkp.tile(
                        [128, CT, 128], FP16, tag="wqk", name=f"w{base}_{jt}"
                    )
                    src = wqT_d[:, base + jt * 128:base + (jt + 1) * 128]
                    nc.gpsimd.dma_start(
                        wg[:], src.rearrange("(ct p) m -> p ct m", p=128)
                    )
                    ps = psA.tile([128, N], F32, tag="ps", name=f"q{base}_{jt}")
                    for ct in range(CT):
                        for qc in range(2):
                            nc.tensor.matmul(
                                ps[:, qc * 512:(qc + 1) * 512],
                                wg[:, ct, :],
                                xqk[:, ct, qc * 512:(qc + 1) * 512],
                                start=(ct == 0),
                                stop=(ct == CT - 1),
                            )
                    nc.vector.tensor_copy(dst[:, jt, :], ps[:])

            # proj weights are first needed far later; load them now so the
            # casting DMAs do not delay the startup x/w loads
            nc.sync.dma_start(
                wps[:], wpT_d.rearrange("(ct p) n -> p ct n", p=128)
            )

            # ---- phase D: attention, head pairs, row-packed S ----------
            deferred_norm = []

            def emit_norm(jobs):
                # jobs = per-pair (rc_h0, rc_h1, p): convert each [1, N]
                # reciprocal row to fp16, broadcast to [128, N] with two K=1
                # masked fp16 ones-matmuls per 512-col chunk, then normalize
                # the pair with one fp16 multiply.
                for rcs_, p_ in jobs:
                    rcrs = []
                    for half in range(2):
                        rcr = normp.tile(
                            [1, N], FP16, tag="rcr", name=f"rcr{2 * p_ + half}", bufs=4
                        )
                        nc.vector.tensor_copy(rcr[:], rcs_[half][:])
                        rcrs.append(rcr)
                    bc_ps = psA.tile([128, N], F32, tag="ps", name=f"bcp{p_}")
                    for qc in range(2):
                        for half in range(2):
                            nc.tensor.matmul(
                                bc_ps[:, qc * 512:(qc + 1) * 512],
                                ones_mask[half][:],
                                rcrs[half][:, qc * 512:(qc + 1) * 512],
                                start=(half == 0),
                                stop=(half == 1),
                            )
                    bc = normp.tile([128, N], FP16, tag="bc", name=f"bc{p_}", bufs=1)
                    nc.vector.tensor_copy(bc[:], bc_ps[:])
                    nc.vector.tensor_mul(ctxN[:, p_, :], ctxN[:, p_, :], bc[:])

            # Software pipeline across head pairs: during pair p's S/exp
            # stream (ACT-paced), the PE executes pair p-1's PV matmuls,
            # whose E tiles are already complete. PV then never waits on the
            # in-flight exp, and attention runs at the ACT exp rate.
            def emit_pv(pcps, pes, pp, kt):
                for half in range(2):
                    h = 2 * pp + half
                    for qc in range(2):
                        nc.tensor.matmul(
                            pcps[half][:, qc * 512:(qc + 1) * 512],
                            V[:, kt, h * (D + 1):(h + 1) * (D + 1)],
                            pes[kt][half][:, qc * 512:(qc + 1) * 512],
                            start=(kt == 0),
                            stop=(kt == NT - 1),
                        )

            def emit_evac(pcps, pp, on_act=False):
                # both PSUM-freeing evacs first; the reciprocal of the
                # denominator row is taken straight from PSUM behind them
                # (off ACT so the exp pacer stays clean, and after the evacs
                # so the ctx-bank handover is not delayed). In the drain the
                # exp stream is over, so the evacs go to ACT to run in
                # parallel with the DVE reciprocals.
                for half in range(2):
                    po = half * 64
                    if on_act:
                        nc.scalar.copy(ctxN[po:po + 64, pp, :], pcps[half][0:D, :])
                    else:
                        nc.vector.tensor_copy(
                            ctxN[po:po + 64, pp, :], pcps[half][0:D, :]
                        )
                rcs = []
                for half in range(2):
                    # custom-DVE ops mis-address non-zero base partitions, so
                    # the PSUM den row must be copied to partition 0 first
                    den = normp.tile(
                        [1, N], F32, tag="den", name=f"den{2 * pp + half}", bufs=4
                    )
                    nc.vector.tensor_copy(den[:], pcps[half][D:D + 1, :])
                    rc = normp.tile(
                        [1, N], F32, tag="rc", name=f"rc{2 * pp + half}", bufs=4
                    )
                    nc.vector.reciprocal_approx_fast(rc[:], den[:])
                    rcs.append(rc)
                deferred_norm.append((rcs, pp))

            prev = None
            for p in range(CT):  # 6 head pairs; pair p = heads (2p, 2p+1)
                cps = [
                    psC.tile([D + 1, N], F32, tag="ctx", name=f"ctx{2 * p + i}")
                    for i in range(2)
                ]
                es = []
                for kt in range(NT):
                    sps = [
                        psA.tile([128, N], F32, tag="ps", name=f"s{2 * p + i}_{kt}")
                        for i in range(2)
                    ]
                    # interleave the two 64-row tile_position halves so their
                    # matmuls run concurrently on the row-tiled PE array
                    for qc in range(2):
                        for half in range(2):
                            po = half * 64
                            nc.tensor.matmul(
                                sps[half][:, qc * 512:(qc + 1) * 512],
                                KT[po:po + 64, p, kt * 128:(kt + 1) * 128],
                                QT[po:po + 64, p, qc * 512:(qc + 1) * 512],
                                start=True,
                                stop=True,
                                tile_position=(po, 0),
                            )
                    row = []
                    for half in range(2):
                        h = 2 * p + half
                        e = ep.tile(
                            [128, N], FP16, tag="e", name=f"e{h}_{kt}", bufs=12
                        )
                        nc.scalar.activation(e[:], sps[half][:], EXP, scale=SCALE)
                        row.append(e)
                    es.append(row)
                    if prev is not None:
                        emit_pv(prev[0], prev[1], prev[2], kt)
                    if kt == 1 and deferred_norm:
                        # normalize the pair before last while streams run
                        emit_norm(deferred_norm)
                        deferred_norm = []
                if prev is not None:
                    emit_evac(prev[0], prev[2])
                prev = (cps, es, p)
            # drain: norm the second-to-last pair first (its reciprocals are
            # ready), then PV + evac for the final pair
            emit_norm(deferred_norm)
            deferred_norm = []
            for kt in range(NT):
                emit_pv(prev[0], prev[1], prev[2], kt)
            emit_evac(prev[0], prev[2], on_act=True)
            last = prev[2]

            # ---- phase E: output projection + bias ---------------------
            # ct outer, lo inner: each ctxN stationary chunk is loaded once.
            # The last pair's ct-chunk is accumulated LAST, and its norm
            # (broadcast matmul + multiply) is emitted after nt=0's first
            # chunks, so the PE projects the already-normalized pairs while
            # the last pair's normalization chain completes.
            cts = [ct for ct in range(CT) if ct != last] + [last]
            for nt in range(NT):
                ps = psA.tile([128, N], F32, tag="ps", name=f"po{nt}")
                for i, ct in enumerate(cts[:-1]):
                    for lo, w in ((0, 512), (512, 256)):
                        nc.tensor.matmul(
                            ps[:, lo:lo + w],
                            ctxN[:, ct, nt * 128:(nt + 1) * 128],
                            wps[:, ct, lo:lo + w],
                            start=(i == 0),
                            stop=False,
                        )
                if nt == 0 and deferred_norm:
                    emit_norm(deferred_norm)
                    deferred_norm = []
                for lo, w in ((0, 512), (512, 256)):
                    nc.tensor.matmul(
                        ps[:, lo:lo + w],
                        ctxN[:, last, nt * 128:(nt + 1) * 128],
                        wps[:, last, lo:lo + w],
                        start=False,
                        stop=True,
                    )
                ob = outp.tile([128, C], F32, tag="ob", name=f"ob{nt}")
                nc.vector.tensor_add(ob[:], ps[:, 0:C], bias_sb[:])
                nc.sync.dma_start(out_d[nt * 128:(nt + 1) * 128, :], ob[:])

    nc.compile()
    return nc


_CACHE = {}


def _get_nc():
    if "nc" not in _CACHE:
        _CACHE["nc"] = build()
    return _CACHE["nc"]


def run(inputs, trace=False):
    """Run on hardware; returns (full_output [8,1024,768] f32, BassKernelResults)."""
    nc = _get_nc()
    x = np.asarray(inputs["x"], dtype=np.float32)
    w_qkv = np.asarray(inputs["w_qkv"], dtype=np.float32)
    w_proj = np.asarray(inputs["w_proj"], dtype=np.float32)
    b_proj = np.asarray(inputs["b_proj"], dtype=np.float32)

    xT = np.ascontiguousarray(x.transpose(0, 2, 1)).astype(np.float16)
    wqT = np.ascontiguousarray(w_qkv.T).astype(np.float16)
    wpT = np.ascontiguousarray(w_proj.T).astype(np.float16)
    bias_bc = np.ascontiguousarray(np.broadcast_to(b_proj.reshape(1, C), (128, C)))

    in_maps = [
        {
            "xT": xT[b],
            "wqT": wqT,
            "wpT": wpT,
            "bias_bc": bias_bc,
            "ones_mask": np.kron(np.eye(2), np.ones((1, 64))).astype(np.float16),
        }
        for b in range(N_CORES)
    ]
    res = run_bass_kernel_spmd(nc, in_maps, list(range(N_CORES)), trace=trace)
    out = np.stack([res.results[b]["out"] for b in range(N_CORES)])
    return out, res


def kernel(x, w_qkv, w_proj, b_proj):
    out, _ = run(
        {"x": x, "w_qkv": w_qkv, "w_proj": w_proj, "b_proj": b_proj}, trace=False
    )
    return out



# revision 28
# speedup vs baseline: 1.0705x; 1.0533x over previous
"""Multi-head self-attention (B=8, N=1024, C=768, H=12, D=64) on 8 Trainium2
NeuronCores, batch-parallel (one batch element per core).

Per-core dataflow (activations kept feature-major, "T" = [feature, token]):
  xT [768,1024] --(PE)--> QT,KT [768,1024] (d-major) and V [1024,768+ones]
  S^T[k,q] = KT_h-slice^T x QT_h      (K=d=64; two heads of a pair via
                                       PE row-tiling at partitions 0/64)
  E = exp(S^T * scale) -> fp16        (ACT; no max-subtract: |S*scale| < 9,
                                       so exp < 6e3 fits fp16 with margin)
  ctxU^T[d,q] (+denominator row) = V_ext_h^T x E   (ones column in V gives
                                                    the softmax denominator)
  evac ctxU fast (frees PSUM); one pair behind: reciprocal + masked K=1
  ones-matmul broadcast + one in-place multiply normalizes the pair.
  out[q,o] = ctxN^T-slices^T x wpT + bias(bcast, DVE add)

The qkv projections run float32r (fp22 multiply, fp32 accumulate, 2 PE
cycles/col). Q/K/V/E/ctx/proj-weight storage is fp16 (e5m10): it streams at
1 PE cycle/col like bf16 but with 8x finer mantissa, halving the S and PV
matmul time at ~5e-4 relative error.
"""
import numpy as np

import concourse.bass as bass
import concourse.tile as tile
from concourse import bacc, mybir
from concourse.bass_utils import run_bass_kernel_spmd

N_CORES = 8
N = 1024          # tokens per core (batch element)
C = 768           # model dim
H = 12            # heads
D = 64            # head dim
SCALE = D ** -0.5
NT = N // 128     # 8 token tiles
CT = C // 128     # 6 feature tiles
F32 = mybir.dt.float32
F32R = mybir.dt.float32r
BF16 = mybir.dt.bfloat16
FP16 = mybir.dt.float16
EXP = mybir.ActivationFunctionType.Exp

QK_BF16 = False   # False: keep the Q/K path (C-phase + S matmuls) in fp32r


def _r(ap):
    return ap.bitcast(F32R)


def build():
    nc = bacc.Bacc(
        "TRN2", target_bir_lowering=False, debug=False, num_devices=N_CORES
    )
    xT_d = nc.dram_tensor("xT", [C, N], FP16, kind="ExternalInput").ap()
    wqT_d = nc.dram_tensor("wqT", [C, 3 * C], FP16, kind="ExternalInput").ap()
    wpT_d = nc.dram_tensor("wpT", [C, C], FP16, kind="ExternalInput").ap()
    bias_d = nc.dram_tensor("bias_bc", [128, C], F32, kind="ExternalInput").ap()
    onesr_d = nc.dram_tensor("ones_mask", [2, 128], FP16, kind="ExternalInput").ap()
    out_d = nc.dram_tensor("out", [N, C], F32, kind="ExternalOutput").ap()

    qk_dt = FP16

    with tile.TileContext(nc) as tc:
        with (
            tc.tile_pool(name="big", bufs=1) as big,
            tc.tile_pool(name="wqk", bufs=3) as wqkp,
            tc.tile_pool(name="e", bufs=4) as ep,
            tc.tile_pool(name="outb", bufs=2) as outp,
            tc.tile_pool(name="norm", bufs=2) as normp,
            tc.tile_pool(name="psA", bufs=2, space="PSUM") as psA,
            tc.tile_pool(name="psC", bufs=2, space="PSUM") as psC,
        ):
            # ---- persistent SBUF tensors -------------------------------
            xqk = big.tile([128, CT, N], FP16, name="xqk", tag="xqk")
            wvs = big.tile([128, CT, C], FP16, name="wvs", tag="wvs")
            wps = big.tile([128, CT, C], FP16, name="wps", tag="wps")
            QT = big.tile([128, CT, N], qk_dt, name="QT", tag="QT")
            KT = big.tile([128, CT, N], qk_dt, name="KT", tag="KT")
            V = big.tile([128, NT, H * (D + 1)], FP16, name="V", tag="V")
            ctxN = big.tile([128, CT, N], FP16, name="ctxN", tag="ctxN")
            bias_sb = big.tile([128, C], F32, name="bias_sb", tag="bias")
            ones_mask = [
                big.tile([1, 128], FP16, name=f"ones_mask{i}", tag=f"onesr{i}")
                for i in range(2)
            ]

            # Batched input DMAs: each dma_start costs ~600ns of issue time on
            # its queue engine, so coarse 3D strided transfers beat per-tile
            # loads.  x on the sync queue, wv on the scalar queue (idle at
            # startup) so the two transfers stream in parallel; both split in
            # ct halves so phase B can start early.
            xT_r = xT_d.rearrange("(ct p) n -> p ct n", p=128)
            wv_r = wqT_d[:, 2 * C:3 * C].rearrange("(ct p) n -> p ct n", p=128)
            hh = CT // 2
            nc.sync.dma_start(xqk[:, 0:hh, :], xT_r[:, 0:hh, :])
            nc.scalar.dma_start(wvs[:, 0:hh, :], wv_r[:, 0:hh, :])
            nc.sync.dma_start(xqk[:, hh:CT, :], xT_r[:, hh:CT, :])
            nc.scalar.dma_start(wvs[:, hh:CT, :], wv_r[:, hh:CT, :])
            for i in range(2):
                nc.gpsimd.dma_start(ones_mask[i][:], onesr_d[i:i + 1, :])
            nc.gpsimd.dma_start(bias_sb[:], bias_d[:])
            v_ones = V[:].rearrange("p nt (h e) -> p (nt h) e", e=D + 1)
            nc.gpsimd.memset(v_ones[:, :, D:D + 1], 1.0)

            # ---- phase B: V (token-major, bf16) ------------------------
            for nt in range(NT):
                pv = psA.tile([128, N], F32, tag="ps", name=f"pv{nt}")
                for ct in range(CT):
                    lhsT = xqk[:, ct, nt * 128:(nt + 1) * 128]
                    for lo, w in ((0, 512), (512, 256)):
                        nc.tensor.matmul(
                            pv[:, lo:lo + w],
                            lhsT,
                            wvs[:, ct, lo:lo + w],
                            start=(ct == 0),
                            stop=(ct == CT - 1),
                        )
                vt = V[:, nt, :].rearrange("p (h e) -> p h e", e=D + 1)
                nc.scalar.copy(
                    vt[:, :, 0:D], pv[:, 0:C].rearrange("p (h d) -> p h d", d=D)
                )

            # ---- phase C: QT / KT (feature-major) ----------------------
            # one batched [128, CT, 128] weight DMA per (jt, base) group,
            # issued from the gpsimd queue to keep the sync queue free
            for jt in range(CT):
                for base, dst in ((0, QT), (C, KT)):
                    wg = wqkp.tile(
                        [128, CT, 128], FP16, tag="wqk", name=f"w{base}_{jt}"
                    )
                    src = wqT_d[:, base + jt * 128:base + (jt + 1) * 128]
                    nc.gpsimd.dma_start(
                        wg[:], src.rearrange("(ct p) m -> p ct m", p=128)
                    )
                    ps = psA.tile([128, N], F32, tag="ps", name=f"q{base}_{jt}")
                    for ct in range(CT):
                        for qc in range(2):
                            nc.tensor.matmul(
                                ps[:, qc * 512:(qc + 1) * 512],
                                wg[:, ct, :],
                                xqk[:, ct, qc * 512:(qc + 1) * 512],
                                start=(ct == 0),
                                stop=(ct == CT - 1),
                            )
                    nc.vector.tensor_copy(dst[:, jt, :], ps[:])

            # proj weights are first needed far later; load them now so the
            # casting DMAs do not delay the startup x/w loads
            nc.sync.dma_start(
                wps[:], wpT_d.rearrange("(ct p) n -> p ct n", p=128)
            )

            # ---- phase D: attention, head pairs, row-packed S ----------
            deferred_norm = []

            def prep_half(pcp, h_, on_act=False):
                # den row -> partition-0 tile (custom-DVE recip mis-addresses
                # non-zero base partitions), reciprocal, fp16 cast
                den = normp.tile([1, N], F32, tag="den", name=f"den{h_}", bufs=4)
                if on_act:
                    nc.scalar.copy(den[:], pcp[D:D + 1, :])
                else:
                    nc.vector.tensor_copy(den[:], pcp[D:D + 1, :])
                rc = normp.tile([1, N], F32, tag="rc", name=f"rc{h_}", bufs=4)
                nc.vector.reciprocal_approx_fast(rc[:], den[:])
                rcr = normp.tile([1, N], FP16, tag="rcr", name=f"rcr{h_}", bufs=4)
                nc.vector.tensor_copy(rcr[:], rc[:])
                return rcr

            def emit_norm(jobs):
                # jobs = per-pair ([rcr_h0, rcr_h1], p): broadcast the fp16
                # reciprocal rows to [128, N] with two K=1 masked fp16
                # ones-matmuls per 512-col chunk, then normalize the pair
                # with one fp16 multiply.
                for rcrs, p_ in jobs:
                    bc_ps = psA.tile([128, N], F32, tag="ps", name=f"bcp{p_}")
                    for qc in range(2):
                        for half in range(2):
                            nc.tensor.matmul(
                                bc_ps[:, qc * 512:(qc + 1) * 512],
                                ones_mask[half][:],
                                rcrs[half][:, qc * 512:(qc + 1) * 512],
                                start=(half == 0),
                                stop=(half == 1),
                            )
                    bc = normp.tile([128, N], FP16, tag="bc", name=f"bc{p_}", bufs=1)
                    nc.vector.tensor_copy(bc[:], bc_ps[:])
                    nc.vector.tensor_mul(ctxN[:, p_, :], ctxN[:, p_, :], bc[:])

            # Software pipeline across head pairs: during pair p's S/exp
            # stream (ACT-paced), the PE executes pair p-1's PV matmuls,
            # whose E tiles are already complete. PV then never waits on the
            # in-flight exp, and attention runs at the ACT exp rate.
            def emit_pv(pcps, pes, pp, kt):
                for half in range(2):
                    h = 2 * pp + half
                    for qc in range(2):
                        nc.tensor.matmul(
                            pcps[half][:, qc * 512:(qc + 1) * 512],
                            V[:, kt, h * (D + 1):(h + 1) * (D + 1)],
                            pes[kt][half][:, qc * 512:(qc + 1) * 512],
                            start=(kt == 0),
                            stop=(kt == NT - 1),
                        )

            def emit_evac(pcps, pp):
                # both PSUM-freeing evacs first (off ACT so the exp pacer
                # stays clean, and first so the ctx-bank handover is not
                # delayed), then the per-half reciprocal prep behind them
                for half in range(2):
                    po = half * 64
                    nc.vector.tensor_copy(
                        ctxN[po:po + 64, pp, :], pcps[half][0:D, :]
                    )
                rcrs = [prep_half(pcps[half], 2 * pp + half) for half in range(2)]
                deferred_norm.append((rcrs, pp))

            prev = None
            for p in range(CT):  # 6 head pairs; pair p = heads (2p, 2p+1)
                cps = [
                    psC.tile([D + 1, N], F32, tag="ctx", name=f"ctx{2 * p + i}")
                    for i in range(2)
                ]
                es = []
                for kt in range(NT):
                    sps = [
                        psA.tile([128, N], F32, tag="ps", name=f"s{2 * p + i}_{kt}")
                        for i in range(2)
                    ]
                    # interleave the two 64-row tile_position halves so their
                    # matmuls run concurrently on the row-tiled PE array
                    for qc in range(2):
                        for half in range(2):
                            po = half * 64
                            nc.tensor.matmul(
                                sps[half][:, qc * 512:(qc + 1) * 512],
                                KT[po:po + 64, p, kt * 128:(kt + 1) * 128],
                                QT[po:po + 64, p, qc * 512:(qc + 1) * 512],
                                start=True,
                                stop=True,
                                tile_position=(po, 0),
                            )
                    row = []
                    for half in range(2):
                        h = 2 * p + half
                        e = ep.tile(
                            [128, N], FP16, tag="e", name=f"e{h}_{kt}", bufs=12
                        )
                        nc.scalar.activation(e[:], sps[half][:], EXP, scale=SCALE)
                        row.append(e)
                    es.append(row)
                    if prev is not None:
                        emit_pv(prev[0], prev[1], prev[2], kt)
                    if kt == 1 and deferred_norm:
                        # normalize the pair before last while streams run
                        emit_norm(deferred_norm)
                        deferred_norm = []
                if prev is not None:
                    emit_evac(prev[0], prev[2])
                prev = (cps, es, p)
            # drain: norm the second-to-last pair first (its reciprocals are
            # ready), then the final pair's PV half-major — half 0's evac +
            # reciprocal chain runs on ACT/DVE underneath half 1's matmuls
            emit_norm(deferred_norm)
            deferred_norm = []
            cps5, es5, last = prev
            rcrs5 = []
            for half in range(2):
                h = 2 * last + half
                for kt in range(NT):
                    for qc in range(2):
                        nc.tensor.matmul(
                            cps5[half][:, qc * 512:(qc + 1) * 512],
                            V[:, kt, h * (D + 1):(h + 1) * (D + 1)],
                            es5[kt][half][:, qc * 512:(qc + 1) * 512],
                            start=(kt == 0),
                            stop=(kt == NT - 1),
                        )
                po = half * 64
                nc.scalar.copy(ctxN[po:po + 64, last, :], cps5[half][0:D, :])
                rcrs5.append(prep_half(cps5[half], h, on_act=False))
            deferred_norm.append((rcrs5, last))

            # ---- phase E: output projection + bias ---------------------
            # ct outer, lo inner: each ctxN stationary chunk is loaded once.
            # The last pair's ct-chunk is accumulated LAST, and its norm
            # (broadcast matmul + multiply) is emitted after nt=0's first
            # chunks, so the PE projects the already-normalized pairs while
            # the last pair's normalization chain completes.
            cts = [ct for ct in range(CT) if ct != last] + [last]
            for nt in range(NT):
                ps = psA.tile([128, N], F32, tag="ps", name=f"po{nt}")
                for i, ct in enumerate(cts[:-1]):
                    for lo, w in ((0, 512), (512, 256)):
                        nc.tensor.matmul(
                            ps[:, lo:lo + w],
                            ctxN[:, ct, nt * 128:(nt + 1) * 128],
                            wps[:, ct, lo:lo + w],
                            start=(i == 0),
                            stop=False,
                        )
                if nt == 0 and deferred_norm:
                    emit_norm(deferred_norm)
                    deferred_norm = []
                for lo, w in ((0, 512), (512, 256)):
                    nc.tensor.matmul(
                        ps[:, lo:lo + w],
                        ctxN[:, last, nt * 128:(nt + 1) * 128],
                        wps[:, last, lo:lo + w],
                        start=False,
                        stop=True,
                    )
                ob = outp.tile([128, C], F32, tag="ob", name=f"ob{nt}")
                nc.vector.tensor_add(ob[:], ps[:, 0:C], bias_sb[:])
                nc.sync.dma_start(out_d[nt * 128:(nt + 1) * 128, :], ob[:])

    nc.compile()
    return nc


_CACHE = {}


def _get_nc():
    if "nc" not in _CACHE:
        _CACHE["nc"] = build()
    return _CACHE["nc"]


def run(inputs, trace=False):
    """Run on hardware; returns (full_output [8,1024,768] f32, BassKernelResults)."""
    nc = _get_nc()
    x = np.asarray(inputs["x"], dtype=np.float32)
    w_qkv = np.asarray(inputs["w_qkv"], dtype=np.float32)
    w_proj = np.asarray(inputs["w_proj"], dtype=np.float32)
    b_proj = np.asarray(inputs["b_proj"], dtype=np.float32)

    xT = np.ascontiguousarray(x.transpose(0, 2, 1)).astype(np.float16)
    wqT = np.ascontiguousarray(w_qkv.T).astype(np.float16)
    wpT = np.ascontiguousarray(w_proj.T).astype(np.float16)
    bias_bc = np.ascontiguousarray(np.broadcast_to(b_proj.reshape(1, C), (128, C)))

    in_maps = [
        {
            "xT": xT[b],
            "wqT": wqT,
            "wpT": wpT,
            "bias_bc": bias_bc,
            "ones_mask": np.kron(np.eye(2), np.ones((1, 64))).astype(np.float16),
        }
        for b in range(N_CORES)
    ]
    res = run_bass_kernel_spmd(nc, in_maps, list(range(N_CORES)), trace=trace)
    out = np.stack([res.results[b]["out"] for b in range(N_CORES)])
    return out, res


def kernel(x, w_qkv, w_proj, b_proj):
    out, _ = run(
        {"x": x, "w_qkv": w_qkv, "w_proj": w_proj, "b_proj": b_proj}, trace=False
    )
    return out



# revision 33
# speedup vs baseline: 1.1104x; 1.0372x over previous
"""Multi-head self-attention (B=8, N=1024, C=768, H=12, D=64) on 8 Trainium2
NeuronCores, batch-parallel (one batch element per core).

Per-core dataflow (activations kept feature-major, "T" = [feature, token]):
  xT [768,1024] --(PE)--> QT,KT [768,1024] (d-major) and V [1024,768+ones]
  S^T[k,q] = KT_h-slice^T x QT_h      (K=d=64; two heads of a pair via
                                       PE row-tiling at partitions 0/64)
  E = exp(S^T * scale) -> fp16        (ACT; no max-subtract: |S*scale| < 9,
                                       so exp < 6e3 fits fp16 with margin)
  ctxU^T[d,q] (+denominator row) = V_ext_h^T x E   (ones column in V gives
                                                    the softmax denominator)
  evac ctxU fast (frees PSUM); one pair behind: reciprocal + masked K=1
  ones-matmul broadcast + one in-place multiply normalizes the pair.
  out[q,o] = ctxN^T-slices^T x wpT + bias(bcast, DVE add)

The qkv projections run float32r (fp22 multiply, fp32 accumulate, 2 PE
cycles/col). Q/K/V/E/ctx/proj-weight storage is fp16 (e5m10): it streams at
1 PE cycle/col like bf16 but with 8x finer mantissa, halving the S and PV
matmul time at ~5e-4 relative error.
"""
import numpy as np

import concourse.bass as bass
import concourse.tile as tile
from concourse import bacc, mybir
from concourse.bass_utils import run_bass_kernel_spmd

N_CORES = 8
N = 1024          # tokens per core (batch element)
C = 768           # model dim
H = 12            # heads
D = 64            # head dim
SCALE = D ** -0.5
NT = N // 128     # 8 token tiles
CT = C // 128     # 6 feature tiles
F32 = mybir.dt.float32
F32R = mybir.dt.float32r
BF16 = mybir.dt.bfloat16
FP16 = mybir.dt.float16
EXP = mybir.ActivationFunctionType.Exp

QK_BF16 = False   # False: keep the Q/K path (C-phase + S matmuls) in fp32r


def _r(ap):
    return ap.bitcast(F32R)


def build():
    nc = bacc.Bacc(
        "TRN2", target_bir_lowering=False, debug=False, num_devices=N_CORES
    )
    xT_d = nc.dram_tensor("xT", [C, N], FP16, kind="ExternalInput").ap()
    wqT_d = nc.dram_tensor("wqT", [C, 3 * C], FP16, kind="ExternalInput").ap()
    wpT_d = nc.dram_tensor("wpT", [C, C], FP16, kind="ExternalInput").ap()
    bias_d = nc.dram_tensor("bias_bc", [128, C], F32, kind="ExternalInput").ap()
    onesr_d = nc.dram_tensor("ones_mask", [2, 128], FP16, kind="ExternalInput").ap()
    out_d = nc.dram_tensor("out", [N, C], F32, kind="ExternalOutput").ap()

    qk_dt = FP16

    with tile.TileContext(nc) as tc:
        with (
            tc.tile_pool(name="big", bufs=1) as big,
            tc.tile_pool(name="wqk", bufs=3) as wqkp,
            tc.tile_pool(name="e", bufs=4) as ep,
            tc.tile_pool(name="outb", bufs=2) as outp,
            tc.tile_pool(name="norm", bufs=2) as normp,
            tc.tile_pool(name="psA", bufs=2, space="PSUM") as psA,
            tc.tile_pool(name="psC", bufs=2, space="PSUM") as psC,
        ):
            # ---- persistent SBUF tensors -------------------------------
            xqk = big.tile([128, CT, N], FP16, name="xqk", tag="xqk")
            wvs = big.tile([128, CT, C], FP16, name="wvs", tag="wvs")
            wps = big.tile([128, CT, C], FP16, name="wps", tag="wps")
            QT = big.tile([128, CT, N], qk_dt, name="QT", tag="QT")
            KT = big.tile([128, CT, N], qk_dt, name="KT", tag="KT")
            V = big.tile([128, NT, H * (D + 1)], FP16, name="V", tag="V")
            ctxN = big.tile([128, CT, N], FP16, name="ctxN", tag="ctxN")
            bias_sb = big.tile([128, C], F32, name="bias_sb", tag="bias")
            ones_mask = [
                big.tile([1, 128], FP16, name=f"ones_mask{i}", tag=f"onesr{i}")
                for i in range(2)
            ]

            # Batched input DMAs: each dma_start costs ~600ns of issue time on
            # its queue engine, so coarse 3D strided transfers beat per-tile
            # loads.  x split across the sync and scalar queues, wv on the
            # gpsimd queue, so all three streams run in parallel and phase B
            # can start as early as possible.
            xT_r = xT_d.rearrange("(ct p) n -> p ct n", p=128)
            wv_r = wqT_d[:, 2 * C:3 * C].rearrange("(ct p) n -> p ct n", p=128)
            hh = CT // 2
            nc.sync.dma_start(xqk[:, 0:hh, :], xT_r[:, 0:hh, :])
            nc.scalar.dma_start(xqk[:, hh:CT, :], xT_r[:, hh:CT, :])
            nc.gpsimd.dma_start(wvs[:, 0:hh, :], wv_r[:, 0:hh, :])
            nc.gpsimd.dma_start(wvs[:, hh:CT, :], wv_r[:, hh:CT, :])
            for i in range(2):
                nc.sync.dma_start(ones_mask[i][:], onesr_d[i:i + 1, :])
            nc.sync.dma_start(bias_sb[:], bias_d[:])
            v_ones = V[:].rearrange("p nt (h e) -> p (nt h) e", e=D + 1)
            nc.gpsimd.memset(v_ones[:, :, D:D + 1], 1.0)

            # ---- phase B: V (token-major, bf16) ------------------------
            for nt in range(NT):
                pv = psA.tile([128, N], F32, tag="ps", name=f"pv{nt}")
                for ct in range(CT):
                    lhsT = xqk[:, ct, nt * 128:(nt + 1) * 128]
                    for lo, w in ((0, 512), (512, 256)):
                        nc.tensor.matmul(
                            pv[:, lo:lo + w],
                            lhsT,
                            wvs[:, ct, lo:lo + w],
                            start=(ct == 0),
                            stop=(ct == CT - 1),
                        )
                vt = V[:, nt, :].rearrange("p (h e) -> p h e", e=D + 1)
                nc.scalar.copy(
                    vt[:, :, 0:D], pv[:, 0:C].rearrange("p (h d) -> p h d", d=D)
                )

            # ---- phase C: QT / KT (feature-major) ----------------------
            # one batched [128, CT, 128] weight DMA per (jt, base) group,
            # issued from the gpsimd queue to keep the sync queue free
            for jt in range(CT):
                for base, dst in ((0, QT), (C, KT)):
                    wg = wqkp.tile(
                        [128, CT, 128], FP16, tag="wqk", name=f"w{base}_{jt}"
                    )
                    src = wqT_d[:, base + jt * 128:base + (jt + 1) * 128]
                    nc.gpsimd.dma_start(
                        wg[:], src.rearrange("(ct p) m -> p ct m", p=128)
                    )
                    ps = psA.tile([128, N], F32, tag="ps", name=f"q{base}_{jt}")
                    for ct in range(CT):
                        for qc in range(2):
                            nc.tensor.matmul(
                                ps[:, qc * 512:(qc + 1) * 512],
                                wg[:, ct, :],
                                xqk[:, ct, qc * 512:(qc + 1) * 512],
                                start=(ct == 0),
                                stop=(ct == CT - 1),
                            )
                    nc.vector.tensor_copy(dst[:, jt, :], ps[:])

            # proj weights are first needed far later; load them now so the
            # casting DMAs do not delay the startup x/w loads
            nc.sync.dma_start(
                wps[:], wpT_d.rearrange("(ct p) n -> p ct n", p=128)
            )

            # ---- phase D: attention, head pairs, row-packed S ----------
            deferred_norm = []

            def prep_half(pcp, h_, on_act=False):
                # den row -> partition-0 tile (custom-DVE recip mis-addresses
                # non-zero base partitions), then reciprocal
                den = normp.tile([1, N], F32, tag="den", name=f"den{h_}", bufs=4)
                if on_act:
                    nc.scalar.copy(den[:], pcp[D:D + 1, :])
                else:
                    nc.vector.tensor_copy(den[:], pcp[D:D + 1, :])
                rc = normp.tile([1, N], F32, tag="rc", name=f"rc{h_}", bufs=4)
                nc.vector.reciprocal_approx_fast(rc[:], den[:])
                rcr = normp.tile([1, N], FP16, tag="rcr", name=f"rcr{h_}", bufs=4)
                nc.vector.tensor_copy(rcr[:], rc[:])
                return rcr

            def emit_norm(jobs, direct=False):
                # jobs = per-pair ([rc_h0, rc_h1], p): broadcast the f32
                # reciprocal rows (streamed as f32r, no cast needed) to
                # [128, N] with two K=1 masked ones-matmuls per 512-col
                # chunk, then normalize the pair with one multiply — via an
                # fp16 staging copy normally (releases the PSUM slot fast),
                # or straight from PSUM in the drain (shorter chain).
                for rcs, p_ in jobs:
                    bc_ps = psA.tile([128, N], F32, tag="ps", name=f"bcp{p_}")
                    for qc in range(2):
                        for half in range(2):
                            nc.tensor.matmul(
                                bc_ps[:, qc * 512:(qc + 1) * 512],
                                ones_mask[half][:],
                                rcs[half][:, qc * 512:(qc + 1) * 512],
                                start=(half == 0),
                                stop=(half == 1),
                            )
                    if direct:
                        nc.vector.tensor_mul(ctxN[:, p_, :], ctxN[:, p_, :], bc_ps[:])
                    else:
                        bc = normp.tile(
                            [128, N], FP16, tag="bc", name=f"bc{p_}", bufs=1
                        )
                        nc.vector.tensor_copy(bc[:], bc_ps[:])
                        nc.vector.tensor_mul(ctxN[:, p_, :], ctxN[:, p_, :], bc[:])

            # Software pipeline across head pairs: during pair p's S/exp
            # stream (ACT-paced), the PE executes pair p-1's PV matmuls,
            # whose E tiles are already complete. PV then never waits on the
            # in-flight exp, and attention runs at the ACT exp rate.
            def emit_pv(pcps, pes, pp, kt):
                for half in range(2):
                    h = 2 * pp + half
                    for qc in range(2):
                        nc.tensor.matmul(
                            pcps[half][:, qc * 512:(qc + 1) * 512],
                            V[:, kt, h * (D + 1):(h + 1) * (D + 1)],
                            pes[kt][half][:, qc * 512:(qc + 1) * 512],
                            start=(kt == 0),
                            stop=(kt == NT - 1),
                        )

            def emit_evac(pcps, pp):
                # both PSUM-freeing evacs first (off ACT so the exp pacer
                # stays clean, and first so the ctx-bank handover is not
                # delayed), then the per-half reciprocal prep behind them
                for half in range(2):
                    po = half * 64
                    nc.vector.tensor_copy(
                        ctxN[po:po + 64, pp, :], pcps[half][0:D, :]
                    )
                rcrs = [prep_half(pcps[half], 2 * pp + half) for half in range(2)]
                deferred_norm.append((rcrs, pp))

            prev = None
            for p in range(CT):  # 6 head pairs; pair p = heads (2p, 2p+1)
                cps = [
                    psC.tile([D + 1, N], F32, tag="ctx", name=f"ctx{2 * p + i}")
                    for i in range(2)
                ]
                es = []
                for kt in range(NT):
                    sps = [
                        psA.tile([128, N], F32, tag="ps", name=f"s{2 * p + i}_{kt}")
                        for i in range(2)
                    ]
                    # interleave the two 64-row tile_position halves so their
                    # matmuls run concurrently on the row-tiled PE array
                    for qc in range(2):
                        for half in range(2):
                            po = half * 64
                            nc.tensor.matmul(
                                sps[half][:, qc * 512:(qc + 1) * 512],
                                KT[po:po + 64, p, kt * 128:(kt + 1) * 128],
                                QT[po:po + 64, p, qc * 512:(qc + 1) * 512],
                                start=True,
                                stop=True,
                                tile_position=(po, 0),
                            )
                    row = []
                    for half in range(2):
                        h = 2 * p + half
                        e = ep.tile(
                            [128, N], FP16, tag="e", name=f"e{h}_{kt}", bufs=12
                        )
                        nc.scalar.activation(e[:], sps[half][:], EXP, scale=SCALE)
                        row.append(e)
                    es.append(row)
                    if prev is not None:
                        emit_pv(prev[0], prev[1], prev[2], kt)
                    if kt == 1 and deferred_norm:
                        # normalize the pair before last while streams run
                        emit_norm(deferred_norm)
                        deferred_norm = []
                if prev is not None:
                    emit_evac(prev[0], prev[2])
                prev = (cps, es, p)
            # drain: norm the second-to-last pair first (its reciprocals are
            # ready), then the final pair's PV half-major — half 0's evac +
            # reciprocal chain runs on ACT/DVE underneath half 1's matmuls
            emit_norm(deferred_norm)
            deferred_norm = []
            cps5, es5, last = prev
            rcrs5 = []
            for half in range(2):
                h = 2 * last + half
                for kt in range(NT):
                    for qc in range(2):
                        nc.tensor.matmul(
                            cps5[half][:, qc * 512:(qc + 1) * 512],
                            V[:, kt, h * (D + 1):(h + 1) * (D + 1)],
                            es5[kt][half][:, qc * 512:(qc + 1) * 512],
                            start=(kt == 0),
                            stop=(kt == NT - 1),
                        )
                po = half * 64
                nc.scalar.copy(ctxN[po:po + 64, last, :], cps5[half][0:D, :])
                rcrs5.append(prep_half(cps5[half], h, on_act=False))
            deferred_norm.append((rcrs5, last))

            # ---- phase E: output projection + bias ---------------------
            # ct outer, lo inner: each ctxN stationary chunk is loaded once.
            # The last pair's ct-chunk is accumulated LAST, and its norm
            # (broadcast matmul + multiply) is emitted after nt=0's first
            # chunks, so the PE projects the already-normalized pairs while
            # the last pair's normalization chain completes.
            cts = [ct for ct in range(CT) if ct != last] + [last]
            for nt in range(NT):
                ps = psA.tile([128, N], F32, tag="ps", name=f"po{nt}")
                for i, ct in enumerate(cts[:-1]):
                    for lo, w in ((0, 512), (512, 256)):
                        nc.tensor.matmul(
                            ps[:, lo:lo + w],
                            ctxN[:, ct, nt * 128:(nt + 1) * 128],
                            wps[:, ct, lo:lo + w],
                            start=(i == 0),
                            stop=False,
                        )
                if nt == 0 and deferred_norm:
                    emit_norm(deferred_norm, direct=True)
                    deferred_norm = []
                for lo, w in ((0, 512), (512, 256)):
                    nc.tensor.matmul(
                        ps[:, lo:lo + w],
                        ctxN[:, last, nt * 128:(nt + 1) * 128],
                        wps[:, last, lo:lo + w],
                        start=False,
                        stop=True,
                    )
                ob = outp.tile([128, C], F32, tag="ob", name=f"ob{nt}")
                nc.vector.tensor_add(ob[:], ps[:, 0:C], bias_sb[:])
                nc.sync.dma_start(out_d[nt * 128:(nt + 1) * 128, :], ob[:])

    nc.compile()
    return nc


_CACHE = {}


def _get_nc():
    if "nc" not in _CACHE:
        _CACHE["nc"] = build()
    return _CACHE["nc"]


def run(inputs, trace=False):
    """Run on hardware; returns (full_output [8,1024,768] f32, BassKernelResults)."""
    nc = _get_nc()
    x = np.asarray(inputs["x"], dtype=np.float32)
    w_qkv = np.asarray(inputs["w_qkv"], dtype=np.float32)
    w_proj = np.asarray(inputs["w_proj"], dtype=np.float32)
    b_proj = np.asarray(inputs["b_proj"], dtype=np.float32)

    xT = np.ascontiguousarray(x.transpose(0, 2, 1)).astype(np.float16)
    wqT = np.ascontiguousarray(w_qkv.T).astype(np.float16)
    wpT = np.ascontiguousarray(w_proj.T).astype(np.float16)
    bias_bc = np.ascontiguousarray(np.broadcast_to(b_proj.reshape(1, C), (128, C)))

    in_maps = [
        {
            "xT": xT[b],
            "wqT": wqT,
            "wpT": wpT,
            "bias_bc": bias_bc,
            "ones_mask": np.kron(np.eye(2), np.ones((1, 64))).astype(np.float16),
        }
        for b in range(N_CORES)
    ]
    res = run_bass_kernel_spmd(nc, in_maps, list(range(N_CORES)), trace=trace)
    out = np.stack([res.results[b]["out"] for b in range(N_CORES)])
    return out, res


def kernel(x, w_qkv, w_proj, b_proj):
    out, _ = run(
        {"x": x, "w_qkv": w_qkv, "w_proj": w_proj, "b_proj": b_proj}, trace=False
    )
    return out

